# revision 76
# baseline (speedup 1.0000x reference)
"""Mamba block kernel for Trainium2 (8 NeuronCores).

Sharding: batch (2-way) x tensor-parallel over d_inner (4-way).
Core c handles batch c//4 and d_inner channels [(c%4)*512, (c%4+1)*512).
Host folds norm_w into in_proj, pre-adds hidden+residual (bf16), and sums
the 4 TP partial outputs per batch.

Device pipeline per core (one NEFF, phases overlap via Tile scheduling):
  A. RMSNorm of r=hid+res (ACT square-accumulate variance, DVE scale),
     PE-transpose via bf16 identity matmuls -> hT_all [1024, L] bf16;
     A/B-phase weights DMA first, F/G weights stream in during phase B
  B. in_proj x-half (k-outer bf16 matmuls, 512-wide moving), causal
     depthwise conv as 4 shifted diag-matmuls on PE accumulating in PSUM
     (SiLU fused in the ACT eviction), x_proj partials
  D. ReduceScatter+AllGather of bf16 x_dbl partials (groups [[0-3],
     [4-7]], DRAM bounce; SBUF collectives are broken in this stack, and
     the RS+AG pair beats one AllReduce's 1.875x cost); the z-half of
     in_proj + SiLU runs under the collective's latency; B and C rows are
     re-read from the bounce 8x-replicated via one merged DMA per block
  F. all four dt preps run at F start (Exps batched before in-place Lns:
     phase F+G then pays only ~two ACT table swaps):
       dt = Ln(1+Exp(dt_proj+bias)) kept bf16 only -- the a = exp(dt*A)
       error then scales with |dt*A| (benign) and ub gets the 2x DVE mode
       ub = dt*x -> one DRAM store per d in (i,s,t) order -> quad
       expansion via 8 partition-broadcast DMAs per quad ([1,4L]->[16,4L]);
       d0's store and quad-0 reads are emitted before d1-3's preps so the
       F lead-in HWDGE convoy stays short
     selective scan, software-pipelined over 64 subtiles [128=(8d x 16n), L]
     (S0 at step, b at step-1, scan at step-3, sel-mm at -4, yg at -5):
       a = exp(dt*A): PE bf16 replicate-matmuls through a 4-deep
         [128,512] PSUM ring (half-bank granularity decouples PE from the
         ACT Exp evictions -- this ring is what paces phase F), A applied
         via the per-partition ACT scale column
       b = ub_exp*B broadcast (DVE bf16 2x)
       h = tensor_tensor_scan on DVE (1x, the hard floor; f32 a required:
         bf16 a would quantize decays near 1 catastrophically)
       hc = h*C (Pool TensorTensor at 0.42 eff; DVE for subtiles 5,13 of
         each chunk balances the engines; Pool cannot touch PSUM or run
         scans/stt, so plain SBUF muls are all it can take)
       y = sel 0/1-matmuls accumulating 16 subtiles into one PSUM tile;
       the last accumulate adds diag(D)*x so no separate y2 pass
       yg = ypsum*silu(z) eviction (DVE), deferred one step to avoid
       head-of-line blocking the next chunk's DVE work
  G. out_proj partial (yg-block stationary, wout moving) -> [L, 1024] f32
"""

import sys

sys.path.insert(0, "/opt/trn_rl_repo")

import numpy as np

import concourse.bacc as bacc
import concourse.tile as tile
from concourse import mybir
from concourse.bass_utils import run_bass_kernel_spmd

F32 = mybir.dt.float32
F32R = mybir.dt.float32r
BF16 = mybir.dt.bfloat16
AF = mybir.ActivationFunctionType
OP = mybir.AluOpType

D_MODEL = 1024
D_INNER = 2048
NST = 16          # d_state
DT_RANK = 64
DCONV = 4
BATCH = 2
L = 2048
EPS = 1e-5

N_CORES = 8
TPG = 4                    # tensor-parallel group size
DLOC = D_INNER // TPG      # 512 channels per core
DC = DLOC // 128           # 4 partition chunks of x-channels
KC = D_MODEL // 128        # 8 contraction chunks
RT = L // 128              # 16 row tiles
NSUB = 128 // NST          # 8 d-channels per expanded tile
SPC = 128 // NSUB          # 16 subtiles per d-chunk

# hc-mul engine assignment: subtile g goes to Pool unless g % SPC in HC_DVE
# (late-in-chunk picks keep Pool fed at d-chunk boundaries)
HC_DVE = frozenset({5, 13})


def _build():
    nc = bacc.Bacc("TRN2", target_bir_lowering=False, debug=False,
                   enable_asserts=True, num_devices=N_CORES)

    def din(name, shape, dt=F32):
        return nc.dram_tensor(name, shape, dt, kind="ExternalInput").ap()

    rin = din("rin", [L, D_MODEL], BF16)        # hid+res, host-added
    winx = din("winx", [D_MODEL, DLOC], BF16)   # in_proj_w[x-slice].T * nw
    winz = din("winz", [D_MODEL, DLOC], BF16)   # in_proj_w[z-slice].T * nw
    wxT = din("wxT", [DLOC, 96], BF16)          # x_proj_w[:, slice].T
    wdtT = din("wdtT", [DT_RANK, DLOC], BF16)   # dt_proj_w[slice].T
    woutT = din("woutT", [DLOC, D_MODEL], BF16)  # out_proj_w[:, slice].T
    convd = din("convd", [128, DC * DCONV * 128], BF16)  # diag stationaries
    dpard = din("dpard", [128, DC * 128], BF16)  # diag(D) stationaries
    convb = din("convb", [128, DC])
    dtb = din("dtb", [128, DC])
    a_sc = din("a_sc", [128, DC * SPC])         # per-tile A scale column
    selm = din("selm", [128, SPC * 128], BF16)  # 16 selection matrices
    expm = din("expm", [128, SPC * 128], BF16)  # 16 expansion matrices
    identb = din("identb", [128, 128], BF16)

    out_part = nc.dram_tensor("out_part", [L, D_MODEL], F32,
                              kind="ExternalOutput").ap()

    with tile.TileContext(nc) as tc:
        cst = tc.alloc_tile_pool(name="cst", bufs=1)
        dram = tc.alloc_tile_pool(name="dram", bufs=1, space="DRAM")
        pW = tc.alloc_tile_pool(name="pW", bufs=1)
        pCv = tc.alloc_tile_pool(name="pCv", bufs=1)

        # ---- constants / weights to SBUF ----
        # A/B-phase weights first so early HWDGE slots feed the first
        # in_proj window; F/G-phase weights stream in during phase B
        convd_sb = pCv.tile([128, DC * DCONV * 128], BF16)
        id_sb = pCv.tile([128, 128], BF16)
        convb_sb = cst.tile([128, DC], F32)
        wx_sb = [pCv.tile([128, 96], BF16, tag=f"wx{d}", name=f"wx{d}")
                 for d in range(DC)]
        eps_sb = cst.tile([128, 1], F32)
        nc.vector.memset(eps_sb[:], EPS)
        dpard_sb = cst.tile([128, DC * 128], BF16)
        dtb_sb = cst.tile([128, DC], F32)
        asc_sb = cst.tile([128, DC * SPC], F32)
        sel_sb = cst.tile([128, SPC * 128], BF16)
        exp_sb = cst.tile([128, SPC * 128], BF16)
        wdt_sb = cst.tile([DT_RANK, DLOC], BF16)
        wout_sb = [cst.tile([128, D_MODEL], BF16, tag=f"wo{d}", name=f"wo{d}")
                   for d in range(DC)]
        winx_sb = [pW.tile([128, DLOC], BF16, tag=f"winx{k}", name=f"winx{k}")
                   for k in range(KC)]
        winz_sb = [pW.tile([128, DLOC], BF16, tag=f"winz{k}", name=f"winz{k}")
                   for k in range(KC)]
        hT_all = pW.tile([128, KC * L], BF16)
        hT_v = hT_all[:].rearrange("p (k t) -> p k t", k=KC)

        # ====== Phases A+B interleaved: RMSNorm/transpose windows feed
        # in_proj x windows; conv on PE; x_proj -> AllReduce; z under AR ====
        nc.sync.dma_start(id_sb[:], identb[:])
        for k in range(KC):
            nc.sync.dma_start(winx_sb[k][:], winx[128 * k:128 * (k + 1), :])
        nc.sync.dma_start(convd_sb[:], convd[:])
        nc.sync.dma_start(convb_sb[:], convb[:])
        for d in range(DC):
            nc.sync.dma_start(wx_sb[d][:], wxT[128 * d:128 * (d + 1), :])
        for k in range(KC):
            nc.sync.dma_start(winz_sb[k][:], winz[128 * k:128 * (k + 1), :])
        nc.sync.dma_start(wdt_sb[:], wdtT[:])
        nc.sync.dma_start(dtb_sb[:], dtb[:])
        nc.sync.dma_start(asc_sb[:], a_sc[:])
        nc.sync.dma_start(sel_sb[:], selm[:])
        nc.sync.dma_start(exp_sb[:], expm[:])
        nc.sync.dma_start(dpard_sb[:], dpard[:])
        for d in range(DC):
            nc.sync.dma_start(wout_sb[d][:], woutT[128 * d:128 * (d + 1), :])

        pBC = tc.alloc_tile_pool(name="pBC", bufs=1, side="right")
        zg = [pBC.tile([128, L], BF16, tag=f"zg{d}", name=f"zg{d}")
              for d in range(DC)]
        xb = [pBC.tile([128, L], BF16, tag=f"xb{d}", name=f"xb{d}")
              for d in range(DC)]
        pDE = tc.alloc_tile_pool(name="pDE", bufs=1, side="right")
        pXP = tc.alloc_tile_pool(name="pXP", bufs=1, side="right")
        xdbl_p = pXP.tile([96, L], BF16)
        dtlow = pDE.tile([64, L], BF16)
        bc = pDE.tile([128, 2 * L], BF16)   # [:, 0:L]=B bcast, [:, L:]=C
        pX = tc.alloc_tile_pool(name="pX", bufs=1, side="right")
        xpad = [pX.tile([128, L + DCONV], BF16, tag=f"xpad{d}",
                        name=f"xpad{d}") for d in range(DC)]
        for d in range(DC):
            nc.vector.memset(xpad[d][:, 0:DCONV - 1], 0.0)

        ps_fr = tc.alloc_tile_pool(name="ps_fr", bufs=1, space="PSUM")
        WN = L // 512   # 4 in_proj windows of 512 timesteps

        with tc.tile_pool(name="pA", bufs=4) as pA, \
             tc.tile_pool(name="pA2", bufs=3) as pA2:

            def emit_rt(rt):
                t0 = 128 * rt
                ld = pA.tile([128, D_MODEL], BF16, tag="ld")
                nc.scalar.dma_start(ld[:], rin[t0:t0 + 128, :])
                sq = pA2.tile([128, D_MODEL], BF16, tag="sq", bufs=1)
                st = pA2.tile([128, 1], F32, tag="st")
                nc.scalar.activation(sq[:], ld[:], AF.Square, accum_out=st[:])
                sg = pA2.tile([128, 1], F32, tag="sg")
                nc.scalar.activation(sg[:], st[:], AF.Sqrt,
                                     bias=eps_sb[:], scale=1.0 / D_MODEL)
                rstd = pA2.tile([128, 1], F32, tag="rstd")
                nc.vector.reciprocal(rstd[:], sg[:])
                hrow = pA2.tile([128, D_MODEL], BF16, tag="hrow")
                nc.vector.tensor_scalar_mul(hrow[:], ld[:], rstd[:])
                for c in range(2):
                    pt = ps_fr.tile([128, 512], BF16, tag="tr", bufs=2,
                                    name="pt")
                    for j in range(4):
                        k = 4 * c + j
                        nc.tensor.transpose(pt[:, 128 * j:128 * (j + 1)],
                                            hrow[:, 128 * k:128 * (k + 1)],
                                            id_sb[:])
                    dst = hT_v[:, 4 * c:4 * (c + 1), t0:t0 + 128]
                    psrc = pt[:].rearrange("p (k t) -> p k t", k=4)
                    nc.vector.tensor_copy(dst, psrc)

            def emit_proj_window(w_sb, w, dest_fn):
                for d in range(DC):
                    pm = ps_fr.tile([128, 512], F32, tag=f"px{d}",
                                    name="pm")
                    for k in range(KC):
                        nc.tensor.matmul(
                            pm[:], w_sb[k][:, 128 * d:128 * (d + 1)],
                            hT_v[:, k, 512 * w:512 * (w + 1)],
                            start=(k == 0), stop=(k == KC - 1))
                    dest_fn(d, w, pm)

            def evict_x(d, w, pm):
                o = DCONV - 1 + 512 * w
                nc.vector.tensor_copy(xpad[d][:, o:o + 512], pm[:])

            def emit_conv_w(d, w):
                # causal conv window: xpad cols [512w, 512w+512+3) suffice
                pm = ps_fr.tile([128, 512], F32, tag="pxp", bufs=2,
                                name="pm")
                for k in range(DCONV):
                    o = 512 * w + k
                    nc.tensor.matmul(
                        pm[:],
                        convd_sb[:, 128 * (DCONV * d + k):
                                 128 * (DCONV * d + k + 1)],
                        xpad[d][:, o:o + 512],
                        start=(k == 0), stop=(k == DCONV - 1))
                nc.scalar.activation(xb[d][:, 512 * w:512 * (w + 1)],
                                     pm[:], AF.Silu,
                                     bias=convb_sb[:, d:d + 1])

            def emit_xproj_w(w):
                pm = ps_fr.tile([128, 512], F32, tag="pxp", bufs=2,
                                name="pm")
                for d in range(DC):
                    nc.tensor.matmul(pm[0:96, :], wx_sb[d][:],
                                     xb[d][:, 512 * w:512 * (w + 1)],
                                     start=(d == 0), stop=(d == DC - 1))
                nc.vector.tensor_copy(xdbl_p[:, 512 * w:512 * (w + 1)],
                                      pm[0:96, :])

            for w in range(WN):
                for rt in range(4 * w, 4 * w + 4):
                    emit_rt(rt)
                emit_proj_window(winx_sb, w, evict_x)
                for d in range(DC):
                    emit_conv_w(d, w)
                emit_xproj_w(w)
        pX.release()

        # ====== Phase D: AllReduce (bf16); z-half fills the latency ======
        bounce_i = dram.tile([96, L], BF16)
        bounce_m = dram.tile([24, L], BF16)
        bounce_o = dram.tile([96, L], BF16)
        nc.sync.dma_start(bounce_i[:], xdbl_p[:])
        pXP.release()
        pCv.release()
        # RS+AG instead of AllReduce: same result, cheaper collective pair
        nc.gpsimd.collective_compute(
            "ReduceScatter", OP.add,
            replica_groups=[[0, 1, 2, 3], [4, 5, 6, 7]],
            ins=[bounce_i.opt()], outs=[bounce_m.opt()])
        nc.gpsimd.collective_compute(
            "AllGather", OP.bypass,
            replica_groups=[[0, 1, 2, 3], [4, 5, 6, 7]],
            ins=[bounce_m.opt()], outs=[bounce_o.opt()])
        nc.sync.dma_start(dtlow[:], bounce_o[0:64, :])
        # B and C rows replicated 8x across partition blocks, one merged
        # DMA per block: dst [16, 2L] <- src rows (64+n | 80+n)
        bc_src = bounce_o[64:96, :].rearrange("(h n) t -> n h t", h=2)
        for i in range(NSUB):
            dst = bc[NST * i:NST * (i + 1), :].rearrange(
                "p (h t) -> p h t", h=2)
            nc.sync.dma_start(dst, bc_src)

        def evict_z(d, w, pm):
            nc.scalar.activation(zg[d][:, 512 * w:512 * (w + 1)], pm[:],
                                 AF.Silu)

        with tc.tile_pool(name="pZ", bufs=1) as _pz:
            for w in range(WN):
                for d in range(DC):
                    pm = ps_fr.tile([128, 512], F32, tag=f"px{d}", name="pm")
                    for k in range(KC):
                        nc.tensor.matmul(
                            pm[:], winz_sb[k][:, 128 * d:128 * (d + 1)],
                            hT_v[:, k, 512 * w:512 * (w + 1)],
                            start=(k == 0), stop=(k == KC - 1))
                    evict_z(d, w, pm)

        pW.release()
        ps_fr.release()

        # ====== Phase F: dt path + selective scan (fused per d) ======
        pY = tc.alloc_tile_pool(name="pY", bufs=1, side="right")
        yg = [pY.tile([128, L], BF16, tag=f"yg{d}", name=f"yg{d}")
              for d in range(DC)]
        with tc.tile_pool(name="pF", bufs=2) as pF, \
             tc.tile_pool(name="pQ", bufs=2) as pQ, \
             tc.tile_pool(name="ps_y", bufs=1, space="PSUM") as ps_y, \
             tc.tile_pool(name="ps_f", bufs=2, space="PSUM") as ps_f:
            dt_ds = {}
            ub_qs = {}
            quads = {}
            a_ts = {}
            hc_ts = {}
            ypsums = {}

            # dt = softplus(dt_proj + bias) = Ln(1 + Exp(.)), stored bf16
            # only: the a = exp(dt*A) error then scales with |dt*A|
            # (benign), and ub gets the 2x DVE mode. All four preps run at
            # F start, Exps batched before Lns, so phase F pays only two
            # ACT table swaps total.
            u_bfs = {}

            def emit_prep_mm(d):
                u_bf = pF.tile([128, L], BF16, tag="u_bf", bufs=4,
                               name="u_bf")
                for q in range(4):
                    o = 512 * q
                    pm = ps_f.tile([128, 512], F32, tag="pa", name="pa", bufs=4)
                    nc.tensor.matmul(
                        pm[:], wdt_sb[:, 128 * d:128 * (d + 1)],
                        dtlow[:, o:o + 512], start=True, stop=True,
                        skip_group_check=True)
                    nc.scalar.activation(u_bf[:, o:o + 512],
                                         pm[:], AF.Exp,
                                         bias=dtb_sb[:, d:d + 1])
                u_bfs[d] = u_bf

            def emit_prep_ub(d):
                # in-place Ln turns the u ring into the dt ring (bf16)
                dt_bf = u_bfs.pop(d)
                nc.scalar.activation(dt_bf[:], dt_bf[:], AF.Ln, bias=1.0)
                dt_ds[d] = dt_bf
                ub_d = pF.tile([128, L], BF16, tag="ub_d", bufs=2, name="ub_d")
                nc.vector.tensor_mul(ub_d[:], dt_bf[:], xb[d][:])
                # DRAM stores in (i, s, t) order for the partition-broadcast
                # expansion reads; one store per quad so the first expansion
                # can start before the whole d-chunk is written
                ub_q = dram.tile([NSUB, SPC * L], BF16, tag="ub_q", bufs=4,
                                 name="ub_q")
                qv = ub_q[:].rearrange("i (s t) -> s i t", s=SPC)
                nc.sync.dma_start(qv, ub_d[:])
                ub_qs[d] = ub_q

            emit_prep_mm(0)

            NG = DC * SPC  # 64 global subtiles

            def emit_quad_reads(qg):
                # quad qg covers subtiles 4qg..4qg+3 of d = qg//4; 8 DMAs,
                # each replicating one (i)-row of the DRAM image to 16
                # partitions ([1, 4L] -> [16, 4L] partition-broadcast)
                quad = pQ.tile([128, 4 * L], BF16, tag="quad", bufs=2)
                ub_q = ub_qs[qg // 4]
                q = qg % 4
                for i in range(NSUB):
                    src = ub_q[i:i + 1, 4 * q * L:4 * (q + 1) * L]
                    eng = nc.sync if i % 2 == 0 else nc.scalar
                    eng.dma_start(quad[NST * i:NST * (i + 1), :],
                                  src.broadcast_to([NST, 4 * L]))
                quads[qg] = quad

            def emit_S0(g):
                # a = exp(dt*A): PE replicate-matmul + ACT Exp, pipelined
                # at half-bank granularity through a 4-deep pa ring
                d, sidx = divmod(g, SPC)
                dt_d = dt_ds[d]
                a_t = pF.tile([128, L], F32, tag="a", bufs=4)
                for q in range(4):
                    o = 512 * q
                    pm = ps_f.tile([128, 512], F32, tag="pa", bufs=4)
                    nc.tensor.matmul(
                        pm[:], exp_sb[:, 128 * sidx:128 * (sidx + 1)],
                        dt_d[:, o:o + 512], start=True, stop=True,
                        skip_group_check=True)
                    nc.scalar.activation(
                        a_t[:, o:o + 512], pm[:], AF.Exp,
                        scale=asc_sb[:, d * SPC + sidx:d * SPC + sidx + 1])
                a_ts[g] = a_t

            b_ts = {}

            def emit_Sb(g):
                # b = ub_exp*B (DVE), one step ahead of the scan
                sq = g % 4
                quad = quads[g // 4]
                b_t = pF.tile([128, L], BF16, tag="b", bufs=4)
                nc.vector.tensor_mul(b_t[:], quad[:, L * sq:L * (sq + 1)],
                                     bc[:, 0:L])
                b_ts[g] = b_t

            def emit_Ss(g):
                # h = scan (DVE), hc = h*C (Pool/DVE)
                h_t = pF.tile([128, L], BF16, tag="h")
                nc.vector.tensor_tensor_scan(h_t[:], a_ts.pop(g)[:],
                                             b_ts.pop(g)[:],
                                             0.0, OP.mult, OP.add)
                hc = pF.tile([128, L], BF16, tag="hc", bufs=4)
                if g % SPC in HC_DVE:
                    nc.vector.tensor_mul(hc[:], h_t[:], bc[:, L:2 * L])
                else:
                    nc.gpsimd.tensor_mul(hc[:], h_t[:], bc[:, L:2 * L])
                hc_ts[g] = hc

            def emit_S4(g):
                d, sidx = divmod(g, SPC)
                if sidx == 0:
                    ypsums[d] = ps_y.tile([128, L], F32, tag="ypsum",
                                          name="ypsum")
                hc = hc_ts.pop(g)
                for tq in range(4):
                    nc.tensor.matmul(
                        ypsums[d][:, 512 * tq:512 * (tq + 1)],
                        sel_sb[:, 128 * sidx:128 * (sidx + 1)],
                        hc[:, 512 * tq:512 * (tq + 1)],
                        start=(sidx == 0), stop=False,
                        skip_group_check=True)
                if sidx == SPC - 1:
                    # fold y += diag(D)*x into the same accumulation
                    for tq in range(4):
                        nc.tensor.matmul(
                            ypsums[d][:, 512 * tq:512 * (tq + 1)],
                            dpard_sb[:, 128 * d:128 * (d + 1)],
                            xb[d][:, 512 * tq:512 * (tq + 1)],
                            start=False, stop=True,
                            skip_group_check=True)

            def emit_S5(g):
                # yg = (y + D*x) * silu(z), one step after the last sel-mm so
                # it does not head-of-line block the next chunk's DVE work
                d, sidx = divmod(g, SPC)
                if sidx != SPC - 1:
                    return
                nc.vector.tensor_mul(yg[d][:], ypsums[d][:], zg[d][:])

            emit_prep_ub(0)
            emit_quad_reads(0)
            for d in range(1, DC):
                emit_prep_mm(d)
            for d in range(1, DC):
                emit_prep_ub(d)
            for step in range(NG + 6):
                g0, gb, gs, g4, g5 = step, step - 1, step - 3, step - 4, \
                    step - 5
                if g0 < NG:
                    gq = g0 + 2
                    if gq % 4 == 0 and gq < NG:
                        emit_quad_reads(gq // 4)
                    emit_S0(g0)
                if 0 <= gb < NG:
                    emit_Sb(gb)
                if 0 <= gs < NG:
                    emit_Ss(gs)
                if 0 <= g5 < NG:
                    emit_S5(g5)
                if 0 <= g4 < NG:
                    emit_S4(g4)
        # ====== Phase G: out_proj ======
        with tc.tile_pool(name="pG", bufs=4) as pG, \
             tc.tile_pool(name="ps_g", bufs=4, space="PSUM") as ps_g:
            for tb in range(RT):
                pm = ps_g.tile([128, D_MODEL], F32, tag="pmG")
                for d in range(DC):
                    for h in range(2):
                        nc.tensor.matmul(
                            pm[:, 512 * h:512 * (h + 1)],
                            yg[d][:, 128 * tb:128 * (tb + 1)],
                            wout_sb[d][:, 512 * h:512 * (h + 1)],
                            start=(d == 0), stop=(d == DC - 1))
                osb = pG.tile([128, D_MODEL], F32, tag="osb")
                nc.scalar.activation(osb[:], pm[:], AF.Copy)
                nc.sync.dma_start(out_part[128 * tb:128 * (tb + 1), :],
                                  osb[:])
        pY.release()
        pDE.release()
        pBC.release()
        cst.release()
        dram.release()
    nc.compile()

    return nc


_NC_CACHE = None


def _get_nc():
    global _NC_CACHE
    if _NC_CACHE is None:
        _NC_CACHE = _build()
    return _NC_CACHE


def kernel(input_ids=None, hidden_states=None, residual=None, norm_w=None,
           in_proj_w=None, conv_w=None, conv_b=None, x_proj_w=None,
           dt_proj_w=None, dt_proj_b=None, A_log=None, D_param=None,
           out_proj_w=None, **kwargs):
    import ml_dtypes
    bf16 = np.dtype(ml_dtypes.bfloat16)

    hs = np.asarray(hidden_states, np.float32)
    rs = np.asarray(residual, np.float32)
    ipw = np.asarray(in_proj_w, np.float32)
    cw = np.asarray(conv_w, np.float32)
    cb = np.asarray(conv_b, np.float32)
    xpw = np.asarray(x_proj_w, np.float32)
    dpw = np.asarray(dt_proj_w, np.float32)
    dpb = np.asarray(dt_proj_b, np.float32)
    al = np.asarray(A_log, np.float32)
    dpr = np.asarray(D_param, np.float32)
    opw = np.asarray(out_proj_w, np.float32)
    nw = np.asarray(norm_w, np.float32)

    r_full = hs + rs                               # host-side residual add

    def colpack(v):  # [DLOC] -> [128, DC], col d = v[d*128:(d+1)*128]
        return np.ascontiguousarray(v.reshape(DC, 128).T).astype(np.float32)

    selm = np.zeros((128, SPC * 128), np.float32)
    expm = np.zeros((128, SPC * 128), np.float32)
    for s in range(SPC):
        for i in range(NSUB):
            m = s * NSUB + i
            for n in range(NST):
                p = i * NST + n
                selm[p, s * 128 + m] = 1.0
                expm[m, s * 128 + p] = 1.0
    identb = np.eye(128, dtype=np.float32)

    nc = _get_nc()
    in_maps = []
    for c in range(N_CORES):
        b, k = c // TPG, c % TPG
        sl = slice(k * DLOC, (k + 1) * DLOC)
        slz = slice(D_INNER + k * DLOC, D_INNER + (k + 1) * DLOC)

        conv4 = cw[sl, 0, :]                       # [DLOC, 4]
        convd = np.zeros((128, DC * DCONV * 128), np.float32)
        for d in range(DC):
            for kk in range(DCONV):
                blk = DCONV * d + kk
                np.fill_diagonal(
                    convd[:, 128 * blk:128 * (blk + 1)],
                    conv4[128 * d:128 * (d + 1), kk])

        dpard = np.zeros((128, DC * 128), np.float32)
        for d in range(DC):
            np.fill_diagonal(dpard[:, 128 * d:128 * (d + 1)],
                             dpr[sl][128 * d:128 * (d + 1)])

        A = -np.exp(al[sl])                        # [DLOC, 16]
        a_sc = np.zeros((128, DC * SPC), np.float32)
        for d in range(DC):
            for s in range(SPC):
                rows = A[d * 128 + s * NSUB: d * 128 + (s + 1) * NSUB, :]
                a_sc[:, d * SPC + s] = rows.reshape(128)

        in_maps.append(dict(
            rin=r_full[b].astype(bf16),
            winx=np.ascontiguousarray(ipw[sl].T * nw[:, None]).astype(bf16),
            winz=np.ascontiguousarray(ipw[slz].T * nw[:, None]).astype(bf16),
            wxT=np.ascontiguousarray(xpw[:, sl].T).astype(bf16),
            wdtT=np.ascontiguousarray(dpw[sl].T).astype(bf16),
            woutT=np.ascontiguousarray(opw[:, sl].T).astype(bf16),
            convd=convd.astype(bf16),
            dpard=dpard.astype(bf16),
            convb=colpack(cb[sl]),
            dtb=colpack(dpb[sl]),
            a_sc=a_sc,
            selm=selm.astype(bf16),
            expm=expm,
            identb=identb.astype(bf16),
        ))

    res = run_bass_kernel_spmd(nc, in_maps, core_ids=list(range(N_CORES)))
    outs = [res.results[c]["out_part"] for c in range(N_CORES)]
    full = np.stack([
        sum(outs[b * TPG + k] for k in range(TPG)) for b in range(BATCH)
    ]).astype(np.float32)
    return full
